# revision 8
# baseline (speedup 1.0000x reference)
"""Causal attention (B=4, S=4096, D_IN=768, D_OUT=64) on 8 Trainium2 NeuronCores.

Sharding: core c handles batch b=c//2 and key-parity p=c%2 (the even or odd
128-wide key tiles of that batch). Every core computes, for ALL queries of its
batch, the unnormalized attention partials over its own key set:
    num[o, q] = sum_{k in own} exp(q.k/8) * V[k, o]
    den[q]    = sum_{k in own} exp(q.k/8)
The host sums the two partials per batch and normalizes: ctx = (num/den).T.
Causality is exact: key-tile work is skipped below the diagonal band and the
boundary blocks are masked with host-provided mask tiles.

Schedule (all bf16 on-chip, fp32 PSUM). Two serial resources matter:
the Scalar/ACT engine (~34us of exp streaming at 1.2GHz) and the PE
(~36us of matmul streaming at 2.4GHz -- but only 1.2GHz unless it has
run gap-free for 3us, so every stall is paid twice). The schedule:
 - x arrives via 4 DMA rings (scalar/gpsimd/vector/sync) with block 0 as
   two parallel halves and the weights split [KV|QQ] so the first
   projection matmul waits only on twKV + half 0.
 - dummy matmuls bridge the PE from program start to the first data
   arrival so the p-state ramp is complete when real work starts.
 - block-0 projections run per half as each half lands; kp/vts/qts
   copies are split between Vector and GpSimd.
 - the main loop emits one scores-group + one exp per "slot" and uses a
   filler queue (ctx drains of the previous tile, projection units for
   upcoming blocks) to pad the PE between slots, so the ACT stream
   never waits and the PE never idles.
 - scores matmuls run as CONCURRENT K=64 pairs on disjoint PE row
   groups (kp[i][0:64] = K^T of key tile 2i, kp[i][64:128] = tile 2i+1;
   Wq is sent duplicated so Q^T exists at partitions 0..63 AND 64..127).
 - ctx matmuls stay M=65 (64 V columns + ones column -> denominator
   free).
 - the last tile's ctx is drained in two column halves so the output
   copy+DMA of the first half overlaps the second half's matmuls.
"""
import numpy as np

import concourse.bass as bass
import concourse.bacc as bacc
import concourse.tile as tile
from concourse import mybir
from concourse.bass_utils import run_bass_kernel_spmd

B, S, DI, DO = 4, 4096, 768, 64
NCORES = 8
NIC = DI // 128          # 6 contraction chunks
NKT = S // 128           # 32 global key tiles per batch
NOWN = NKT // 2          # 16 own key tiles per core
QT = 512                 # query tile width
NQT = S // QT            # 8 query tiles
ORD = [0, 1, 3, 2, 7, 6, 5, 4]       # query-tile processing order
F32 = mybir.dt.float32
BF16 = mybir.dt.bfloat16
NWARM = 32               # dummy warmup matmuls (PE p-state bridge)

_prog_cache = {}


def j0_of(T):
    """First diagonal-region packed key tile for permuted query tile T."""
    return 4 * T if T < 4 else 4 * (T - 4)


def build_program():
    """Build + compile the single SPMD Bass program (identical on all cores)."""
    nc = bacc.Bacc("TRN2", target_bir_lowering=False, debug=False)

    # x^T relaid by the host to [partition, block, chunk, col]; block 0 is
    # stored [p, half, chunk, 256] so each half is one contiguous DMA.
    xT = nc.declare_dram_parameter("xT", [128, NQT * NIC * QT], BF16,
                                   isOutput=False)
    # [Wk|Wv] then [Wq|Wq] (Wq duplicated so Q^T appears at partitions 0..63
    # AND 64..127), each relaid to [128, chunk, 128] contiguous per partition.
    wall = nc.declare_dram_parameter("wall", [128, 2 * NIC * 128], BF16,
                                     isOutput=False)
    # [mdiag | mpcol | ident(zero-padded)] as one [128, 320] block
    mall = nc.declare_dram_parameter("mall", [128, 320], BF16, isOutput=False)
    nd = nc.declare_dram_parameter("nd", [DO + 1, S], F32, isOutput=True)

    with tile.TileContext(nc) as tc:
        with tc.tile_pool(name="consts", bufs=1) as consts, \
             tc.tile_pool(name="xpool", bufs=1) as xpool, \
             tc.tile_pool(name="qkv", bufs=1) as qkv, \
             tc.tile_pool(name="expp", bufs=10) as expp, \
             tc.tile_pool(name="ndst", bufs=2) as ndst, \
             tc.tile_pool(name="ps_sc", bufs=2, space="PSUM") as ps_sc, \
             tc.tile_pool(name="ps_pj", bufs=1, space="PSUM") as ps_pj, \
             tc.tile_pool(name="ps_ctx", bufs=1, space="PSUM") as ps_ctx:

            BW = NIC * QT  # 3072 cols per x block
            HB = BW // 2
            # ---- input DMAs, issued first thing.  Only sync/scalar/gpsimd
            # queues can start DMAs; each ring executes its transfers in
            # order, so ring order == priority order and no gating is needed.
            #   sync:   twKV, twQ, tm, xb1, xb7   (+ nd outputs later)
            #   scalar: xb0 half0 (then stays free for the exp stream)
            #   gpsimd: xb0 half1, xb3, xb2, xb6, xb5, xb4
            xb = [None] + [xpool.tile([128, BW], BF16, tag=f"xb_{cb}",
                                      name=f"xb_{cb}")
                           for cb in range(1, NQT)]
            xb0h = [xpool.tile([128, HB], BF16, tag=f"xb0h{h}", name=f"xb0h{h}")
                    for h in range(2)]
            twKV = consts.tile([128, NIC, 128], BF16, tag="twKV", name="twKV")
            twQ = consts.tile([128, NIC, 128], BF16, tag="twQ", name="twQ")
            tm = consts.tile([128, 320], BF16, tag="tm", name="tm")

            nc.sync.dma_start(out=twKV, in_=wall[:, 0:NIC * 128])
            nc.scalar.dma_start(out=xb0h[0], in_=xT[:, 0:HB])
            nc.gpsimd.dma_start(out=xb0h[1], in_=xT[:, HB:BW])
            nc.sync.dma_start(out=twQ, in_=wall[:, NIC * 128:2 * NIC * 128])
            nc.sync.dma_start(out=tm, in_=mall[:, :])
            nc.sync.dma_start(out=xb[1], in_=xT[:, BW:2 * BW])
            nc.sync.dma_start(out=xb[7], in_=xT[:, 7 * BW:8 * BW])
            for cb in (3, 2, 6, 5, 4):
                nc.gpsimd.dma_start(out=xb[cb],
                                    in_=xT[:, cb * BW:(cb + 1) * BW])

            tmd = tm[:, 0:128]
            tmp = tm[:, 128:256]
            tid = tm[0:DO, 256:320]

            # ---- PE p-state bridge: dummy matmuls from program start until
            # the first x data lands, so the 3us continuous-execution ramp is
            # complete when real work starts.
            dum = consts.tile([128, 128], BF16, tag="dum", name="dum")
            nc.vector.memset(dum, 0.0)
            pdum = ps_sc.tile([128, 3 * QT], F32, tag="psc", name="psc")
            for _ in range(NWARM):
                nc.tensor.matmul(pdum[:, 0:128], dum, dum, start=True, stop=True)

            zsrc = consts.tile([DO, 1], F32, tag="zsrc", name="zsrc")
            nc.vector.memset(zsrc, 0.0)
            # Dummy exp pulls the ~1.3us ACT table load off the critical path.
            zexp = consts.tile([DO, 1], F32, tag="zexp", name="zexp")
            nc.scalar.activation(zexp, zsrc,
                                 mybir.ActivationFunctionType.Exp, scale=1.0)

            def xc(ic, cb):
                return xb[cb][:, ic * QT:(ic + 1) * QT]

            def xc0(half, ic):
                return xb0h[half][:, ic * 256:(ic + 1) * 256]

            # ---- projection state ----
            # kp[i]: K^T of key tile 2i at partitions 0..63, tile 2i+1 at
            # 64..127
            kps = [qkv.tile([128, 128], BF16, tag=f"kp_{i}", name=f"kp_{i}")
                   for i in range(NOWN // 2)]
            vts = [qkv.tile([DO, QT], BF16, tag=f"vt_{st}", name=f"vt_{st}")
                   for st in range(4)]
            qts = [qkv.tile([128, QT], BF16, tag=f"qt_{st}", name=f"qt_{st}")
                   for st in range(NQT)]
            # all V1 tiles in one buffer: [128 keys, key tile, 64 V cols + 1s]
            v1big = qkv.tile([128, NOWN, DO + 1], BF16, tag="v1big",
                             name="v1big")
            nc.vector.memset(v1big[:, :, DO:DO + 1], 1.0)

            def v1(j):
                return v1big[:, j, :]

            def kv_units(st):
                """K/V projection of own key column block st, as small PE
                units; copies split across Vector and GpSimd."""
                p1 = ps_pj.tile([128, QT], F32, tag="pspj", name="pspj")
                for ic in range(0, NIC, 2):
                    def mm2(ic=ic, p1=p1):
                        nc.tensor.matmul(p1, twKV[:, ic, :], xc(ic, st),
                                         start=(ic == 0), stop=False)
                        nc.tensor.matmul(p1, twKV[:, ic + 1, :], xc(ic + 1, st),
                                         start=False, stop=(ic + 1 == NIC - 1))
                    yield mm2

                def copies(p1=p1):
                    nc.vector.tensor_copy(vts[st], p1[DO:128, :])
                    for u in range(2):
                        kp = kps[2 * st + u]
                        nc.vector.tensor_copy(kp[0:DO, :],
                                              p1[0:DO, 256 * u:256 * u + 128])
                        nc.vector.tensor_copy(kp[DO:128, :],
                                              p1[0:DO, 256 * u + 128:256 * u + 256])
                yield copies

            def tr_unit(st):
                """V transposes for block st -> v1big rows 4st..4st+3."""
                def transp():
                    pvq = ps_pj.tile([128, 4, DO], BF16, tag="pspj", name="pspj")
                    for r in range(4):
                        nc.tensor.transpose(pvq[:, r, :],
                                            vts[st][:, r * 128:r * 128 + 128],
                                            tid)
                    nc.vector.tensor_copy(v1big[:, 4 * st:4 * st + 4, 0:DO], pvq)
                yield transp

            def q_units(st):
                """Q^T (duplicated at partitions 0..63 / 64..127) for block
                st."""
                p2 = ps_pj.tile([128, QT], F32, tag="pspj", name="pspj")
                for ic in range(0, NIC, 2):
                    def mm2(ic=ic, p2=p2):
                        nc.tensor.matmul(p2, twQ[:, ic, :], xc(ic, st),
                                         start=(ic == 0), stop=False)
                        nc.tensor.matmul(p2, twQ[:, ic + 1, :], xc(ic + 1, st),
                                         start=False, stop=(ic + 1 == NIC - 1))
                    yield mm2

                def qcopy(p2=p2):
                    nc.vector.tensor_copy(qts[st], p2)
                yield qcopy

            exp_scale = float(1.0 / np.sqrt(DO))

            def mm_sc(T, j, w, sp, off):
                """One K=64 scores matmul: key tile j x last w queries of tile
                T, into sp[:, off:off+w]. Row-group from j's parity."""
                kp = kps[j // 2]
                lo = DO * (j % 2)
                nc.tensor.matmul(sp[:, off:off + w], kp[lo:lo + DO, :],
                                 qts[T][lo:lo + DO, QT - w:QT],
                                 start=True, stop=True)

            class CtxDrain:
                """Phase B for a query tile, drained a few matmuls at a time
                via the filler queue so ctx work interleaves between the next
                tile's scores groups in the in-order PE queue."""

                def __init__(self, T, ctx_args):
                    self.T = T
                    self.nk = j0_of(T) + 4
                    self.args = ctx_args
                    self.i = 0
                    self.ctxp = ps_ctx.tile([DO + 1, QT], F32, tag="ctxp",
                                            name="ctxp")

                def drain(self, n):
                    while self.i < len(self.args) and n > 0:
                        j, et_ap, qlo, w = self.args[self.i]
                        nc.tensor.matmul(self.ctxp[:, qlo:QT], v1(j),
                                         et_ap[:, 0:w],
                                         start=(j == 0), stop=(j == self.nk - 1))
                        self.i += 1
                        n -= 1

                def finish(self):
                    self.drain(len(self.args))
                    ost = ndst.tile([DO + 1, QT], F32, tag="ost", name="ost")
                    nc.vector.tensor_copy(ost, self.ctxp)
                    nc.sync.dma_start(out=nd[:, self.T * QT:(self.T + 1) * QT],
                                      in_=ost)

            # ---- block-0 projections, per half: each half's matmul chain,
            # then its kp/vts/qts copies, start as soon as that half lands.
            p1 = ps_pj.tile([128, QT], F32, tag="pspj", name="pspj")
            p2 = ps_sc.tile([128, 3 * QT], F32, tag="psc", name="psc")
            for half in range(2):
                for ic in range(NIC):
                    nc.tensor.matmul(p1[:, half * 256:half * 256 + 256],
                                     twKV[:, ic, :], xc0(half, ic),
                                     start=(ic == 0), stop=(ic == NIC - 1))
                for ic in range(NIC):
                    nc.tensor.matmul(p2[:, half * 256:half * 256 + 256],
                                     twQ[:, ic, :], xc0(half, ic),
                                     start=(ic == 0), stop=(ic == NIC - 1))
                kp = kps[half]
                nc.vector.tensor_copy(kp[0:DO, :],
                                      p1[0:DO, 256 * half:256 * half + 128])
                nc.vector.tensor_copy(
                    kp[DO:128, :],
                    p1[0:DO, 256 * half + 128:256 * half + 256])
                nc.vector.tensor_copy(vts[0][:, 256 * half:256 * half + 256],
                                      p1[DO:128, 256 * half:256 * half + 256])
                nc.vector.tensor_copy(qts[0][:, 256 * half:256 * half + 256],
                                      p2[:, 256 * half:256 * half + 256])

            # ---- main loop ----
            # fillers: FIFO of (deadline, closure) proj units.  Deadlines are
            # 2*pos for units a position's full-tile scores / first ctx
            # drains depend on, 2*pos+1 for units only its band depends on.
            # The queue is pushed in nondecreasing deadline order, so a
            # front-pop flush is sufficient.
            fillers = []

            def push_units(gen, dl):
                for u in gen:
                    fillers.append((dl, u))

            def run_fillers(n):
                k = 0
                while fillers and k < n:
                    fillers.pop(0)[1]()
                    k += 1

            def flush(dl_max):
                while fillers and fillers[0][0] <= dl_max:
                    fillers.pop(0)[1]()

            # projection units per position: (st, kind, deadline).
            PROJ = {
                0: [(1, "q", 2), (0, "tr", 2), (1, "kv", 3), (1, "tr", 4)],
                1: [(3, "q", 4), (2, "kv", 4), (3, "kv", 5)],
                2: [(2, "q", 6), (3, "tr", 6), (2, "tr", 6)],
                3: [(7, "q", 8)],
                4: [(6, "q", 10)],
                5: [(5, "q", 12)],
                6: [(4, "q", 14)],
                7: [],
            }

            pending = None  # CtxDrain from the previous iteration
            for pos in range(NQT):
                T = ORD[pos]
                j0 = j0_of(T)
                mask = tmd if T < 4 else tmp
                ctx_args = []   # (j, et_ap, qlo, w) consumed via fillers

                for st, kind, dl in PROJ[pos]:
                    if kind == "q":
                        push_units(q_units(st), dl)
                    elif kind == "kv":
                        push_units(kv_units(st), dl)
                    else:
                        push_units(tr_unit(st), dl)
                # correctness: everything this position's full-tile scores or
                # first ctx drains depend on must already be emitted.
                flush(2 * pos)

                # pace leftovers + prev tile's ctx across this position's
                # full-tile slots
                nslots = -(-j0 // 3)
                per_slot = -(-len(fillers) // nslots) if nslots else 0
                dn = (-(-len(pending.args) // nslots)
                      if pending is not None and nslots else 0)

                # full key tiles 0..j0-1 in triples sharing one 3-bank PSUM
                # tile and one 1536-col exp
                j = 0
                while j < j0:
                    cnt = min(3, j0 - j)
                    sp = ps_sc.tile([128, 3 * QT], F32, tag="psc", name="psc")
                    et = expp.tile([128, 3 * QT], BF16, tag="et", name="et")
                    for u in range(cnt):
                        mm_sc(T, j + u, QT, sp, u * QT)
                    nc.scalar.activation(et[:, 0:cnt * QT], sp[:, 0:cnt * QT],
                                         mybir.ActivationFunctionType.Exp,
                                         scale=exp_scale)
                    for u in range(cnt):
                        ctx_args.append((j + u, et[:, u * QT:(u + 1) * QT],
                                         0, QT))
                    if pending is not None:
                        pending.drain(dn)
                    run_fillers(per_slot)
                    j += cnt
                # band deps (kps of the diagonal tiles)
                flush(2 * pos + 1)
                # diagonal band: all 4 tiles in ONE 3-bank tile / one exp:
                # r0 [0:512] bank1, r1 [512:896] bank2, r3 [896:1024] bank2,
                # r2 [1024:1280] bank3 (concurrent pairs hit distinct banks).
                sp = ps_sc.tile([128, 3 * QT], F32, tag="psc", name="psc")
                et = expp.tile([128, 3 * QT], BF16, tag="et", name="et")
                mm_sc(T, j0, QT, sp, 0)
                mm_sc(T, j0 + 1, 384, sp, QT)
                mm_sc(T, j0 + 2, 256, sp, 2 * QT)
                mm_sc(T, j0 + 3, 128, sp, QT + 384)
                nc.scalar.activation(et[:, 0:2 * QT + 256], sp[:, 0:2 * QT + 256],
                                     mybir.ActivationFunctionType.Exp,
                                     scale=exp_scale)
                # masks split across Vector and GpSimd
                nc.vector.tensor_mul(et[:, 0:128], et[:, 0:128], mask)
                nc.gpsimd.tensor_mul(et[:, QT:QT + 128], et[:, QT:QT + 128],
                                     mask)
                nc.vector.tensor_mul(et[:, QT + 384:2 * QT],
                                     et[:, QT + 384:2 * QT], mask)
                nc.gpsimd.tensor_mul(et[:, 2 * QT:2 * QT + 128],
                                     et[:, 2 * QT:2 * QT + 128], mask)
                ctx_args.append((j0, et[:, 0:QT], 0, QT))
                ctx_args.append((j0 + 1, et[:, QT:QT + 384], 128, 384))
                ctx_args.append((j0 + 2, et[:, 2 * QT:2 * QT + 256], 256, 256))
                ctx_args.append((j0 + 3, et[:, QT + 384:2 * QT], 384, 128))

                if pending is not None:
                    pending.finish()
                if j0 == 0 and pos < NQT - 1:
                    # no full-tile slots at this position: emit the next
                    # position's dependencies now, overlapping the band exp.
                    flush(2 * (pos + 1))
                if pos < NQT - 1:
                    pending = CtxDrain(T, ctx_args)
                else:
                    # ---- tail: drain the last tile's ctx in two column
                    # halves so the first half's copy+DMA overlaps the second
                    # half's matmuls.  j0 == 0 for the last tile (band only).
                    run_fillers(len(fillers))
                    ctxp = ps_ctx.tile([DO + 1, QT], F32, tag="ctxp",
                                       name="ctxp")
                    eb = et
                    H = QT // 2
                    # half A: output cols 0:256 <- tiles j0 (cols 0:256) and
                    # j0+1 (out cols 128:256 = its et cols 0:128)
                    nc.tensor.matmul(ctxp[:, 0:H], v1(0), eb[:, 0:H],
                                     start=True, stop=False)
                    nc.tensor.matmul(ctxp[:, 128:H], v1(1), eb[:, QT:QT + 128],
                                     start=False, stop=True)
                    ostA = ndst.tile([DO + 1, H], F32, tag="ost", name="ostA")
                    nc.vector.tensor_copy(ostA, ctxp[:, 0:H])
                    nc.sync.dma_start(out=nd[:, T * QT:T * QT + H], in_=ostA)
                    # half B: output cols 256:512
                    ctxp2 = ps_pj.tile([DO + 1, H], F32, tag="pspj",
                                       name="ctxp2")
                    nc.tensor.matmul(ctxp2, v1(0), eb[:, H:QT],
                                     start=True, stop=False)
                    nc.tensor.matmul(ctxp2[:, 0:H], v1(1),
                                     eb[:, QT + 128:QT + 384],
                                     start=False, stop=False)
                    nc.tensor.matmul(ctxp2[:, 0:H], v1(2),
                                     eb[:, 2 * QT:2 * QT + 256],
                                     start=False, stop=False)
                    nc.tensor.matmul(ctxp2[:, 128:H], v1(3),
                                     eb[:, QT + 384:2 * QT],
                                     start=False, stop=True)
                    ostB = ndst.tile([DO + 1, H], F32, tag="ost", name="ostB")
                    nc.vector.tensor_copy(ostB, ctxp2)
                    nc.sync.dma_start(out=nd[:, T * QT + H:(T + 1) * QT],
                                      in_=ostB)

    nc.compile()
    return nc


def get_program():
    if "nc" not in _prog_cache:
        _prog_cache["nc"] = build_program()
    return _prog_cache["nc"]


def core_perm(parity):
    """Permuted-to-global column index map: own key tiles first, then other."""
    own = [g for g in range(NKT) if g % 2 == parity]
    other = [g for g in range(NKT) if g % 2 != parity]
    return np.concatenate([np.arange(g * 128, (g + 1) * 128)
                           for g in own + other])


def _to_bf16(a):
    from concourse import mybir as _mybir
    return np.ascontiguousarray(a.astype(_mybir.dt.np(_mybir.dt.bfloat16)))


def make_in_maps(x, Wq, Wk, Wv):
    x = np.asarray(x, dtype=np.float32)
    Wq = np.asarray(Wq, dtype=np.float32)
    Wk = np.asarray(Wk, dtype=np.float32)
    Wv = np.asarray(Wv, dtype=np.float32)
    wkv = np.concatenate([Wk, Wv], axis=1)                     # [768, 128]
    wqq = np.concatenate([Wq, Wq], axis=1)                     # [768, 128]
    wkv_r = wkv.reshape(NIC, 128, 128).transpose(1, 0, 2).reshape(128, -1)
    wqq_r = wqq.reshape(NIC, 128, 128).transpose(1, 0, 2).reshape(128, -1)
    wall = _to_bf16(np.concatenate([wkv_r, wqq_r], axis=1))    # [128, 1536]
    mdiag = np.triu(np.ones((128, 128), dtype=np.float32))
    identp = np.concatenate([np.eye(DO, dtype=np.float32),
                             np.zeros((128 - DO, DO), np.float32)], axis=0)
    in_maps = []
    perms = []
    for c in range(NCORES):
        b, par = c // 2, c % 2
        perm = core_perm(par)
        perms.append(perm)
        xTp = x[b].T[:, perm]                                  # [768, 4096]
        # [p, block, chunk, col] layout, contiguous per partition per block;
        # block 0 is stored [p, half, chunk, 256] so its two column halves
        # are each one contiguous DMA
        blocks = xTp.reshape(NIC, 128, NQT, QT).transpose(1, 2, 0, 3)
        b0 = (blocks[:, 0].reshape(128, NIC, 2, 256).transpose(0, 2, 1, 3)
              .reshape(128, NIC * QT))
        rest = blocks[:, 1:].reshape(128, (NQT - 1) * NIC * QT)
        xr = np.concatenate([b0, rest], axis=1)
        mpcol = np.full((128, 128), 1.0 - par, dtype=np.float32)
        mall = np.concatenate([mdiag, mpcol, identp], axis=1)  # [128, 320]
        in_maps.append({
            "xT": _to_bf16(xr), "wall": wall, "mall": _to_bf16(mall),
        })
    return in_maps, perms


def combine(results, perms):
    out = np.empty((B, S, DO), dtype=np.float32)
    for b in range(B):
        num = np.zeros((DO, S), dtype=np.float64)
        den = np.zeros((S,), dtype=np.float64)
        for c in (2 * b, 2 * b + 1):
            nd_c = results[c]["nd"].astype(np.float64)
            inv = np.empty(S, dtype=np.int64)
            inv[perms[c]] = np.arange(S)
            nd_g = nd_c[:, inv]
            num += nd_g[:DO]
            den += nd_g[DO]
        out[b] = (num / den).T.astype(np.float32)
    return out


def kernel(x, Wq, Wk, Wv):
    nc = get_program()
    in_maps, perms = make_in_maps(x, Wq, Wk, Wv)
    res = run_bass_kernel_spmd(nc, in_maps, list(range(NCORES)))
    return combine(res.results, perms)


# revision 10
# speedup vs baseline: 1.0649x; 1.0649x over previous
"""Causal attention (B=4, S=4096, D_IN=768, D_OUT=64) on 8 Trainium2 NeuronCores.

Sharding: core c handles batch b=c//2 and key-parity p=c%2 (the even or odd
128-wide key tiles of that batch). Every core computes, for ALL queries of its
batch, the unnormalized attention partials over its own key set:
    num[o, q] = sum_{k in own} exp(q.k/8) * V[k, o]
    den[q]    = sum_{k in own} exp(q.k/8)
The host sums the two partials per batch and normalizes: ctx = (num/den).T.
Causality is exact: key-tile work is skipped below the diagonal band and the
boundary blocks are masked with host-provided mask tiles.

Schedule (all bf16 on-chip, fp32 PSUM). Two serial resources matter:
the Scalar/ACT engine (~34us of exp streaming at 1.2GHz) and the PE
(~36us of matmul streaming at 2.4GHz -- but only 1.2GHz unless it has
run gap-free for 3us, so every stall is paid twice). The schedule:
 - x arrives via 4 DMA rings (scalar/gpsimd/vector/sync) with block 0 as
   two parallel halves and the weights split [KV|QQ] so the first
   projection matmul waits only on twKV + half 0.
 - dummy matmuls bridge the PE from program start to the first data
   arrival so the p-state ramp is complete when real work starts.
 - block-0 projections run per half as each half lands; kp/vts/qts
   copies are split between Vector and GpSimd.
 - the main loop emits one scores-group + one exp per "slot" and uses a
   filler queue (ctx drains of the previous tile, projection units for
   upcoming blocks) to pad the PE between slots, so the ACT stream
   never waits and the PE never idles.
 - scores matmuls run as CONCURRENT K=64 pairs on disjoint PE row
   groups (kp[i][0:64] = K^T of key tile 2i, kp[i][64:128] = tile 2i+1;
   Wq is sent duplicated so Q^T exists at partitions 0..63 AND 64..127).
 - ctx matmuls stay M=65 (64 V columns + ones column -> denominator
   free).
 - the last tile's ctx is drained in two column halves so the output
   copy+DMA of the first half overlaps the second half's matmuls.
"""
import numpy as np

import concourse.bass as bass
import concourse.bacc as bacc
import concourse.tile as tile
from concourse import mybir
from concourse.bass_utils import run_bass_kernel_spmd

B, S, DI, DO = 4, 4096, 768, 64
NCORES = 8
NIC = DI // 128          # 6 contraction chunks
NKT = S // 128           # 32 global key tiles per batch
NOWN = NKT // 2          # 16 own key tiles per core
QT = 512                 # query tile width
NQT = S // QT            # 8 query tiles
ORD = [0, 1, 3, 2, 7, 6, 5, 4]       # query-tile processing order
F32 = mybir.dt.float32
BF16 = mybir.dt.bfloat16
NWARM = 34               # dummy warmup matmuls (PE p-state bridge)

_prog_cache = {}


def j0_of(T):
    """First diagonal-region packed key tile for permuted query tile T."""
    return 4 * T if T < 4 else 4 * (T - 4)


def build_program():
    """Build + compile the single SPMD Bass program (identical on all cores)."""
    nc = bacc.Bacc("TRN2", target_bir_lowering=False, debug=False)

    # x^T relaid by the host to [partition, block, chunk, col]; block 0 is
    # stored [p, half, chunk, 256] so each half is one contiguous DMA.
    xT = nc.declare_dram_parameter("xT", [128, NQT * NIC * QT], BF16,
                                   isOutput=False)
    # [Wk|Wv] then [Wq|Wq] (Wq duplicated so Q^T appears at partitions 0..63
    # AND 64..127), each relaid to [128, chunk, 128] contiguous per partition.
    wall = nc.declare_dram_parameter("wall", [128, 2 * NIC * 128], BF16,
                                     isOutput=False)
    # [mdiag | mpcol | ident(zero-padded)] as one [128, 320] block
    mall = nc.declare_dram_parameter("mall", [128, 320], BF16, isOutput=False)
    nd = nc.declare_dram_parameter("nd", [DO + 1, S], F32, isOutput=True)

    with tile.TileContext(nc) as tc:
        with tc.tile_pool(name="consts", bufs=1) as consts, \
             tc.tile_pool(name="xpool", bufs=1) as xpool, \
             tc.tile_pool(name="qkv", bufs=1) as qkv, \
             tc.tile_pool(name="expp", bufs=10) as expp, \
             tc.tile_pool(name="ndst", bufs=2) as ndst, \
             tc.tile_pool(name="ps_sc", bufs=2, space="PSUM") as ps_sc, \
             tc.tile_pool(name="ps_pj", bufs=1, space="PSUM") as ps_pj, \
             tc.tile_pool(name="ps_ctx", bufs=1, space="PSUM") as ps_ctx:

            BW = NIC * QT  # 3072 cols per x block
            HB = BW // 2
            # ---- input DMAs, issued first thing.  Only sync/scalar/gpsimd
            # queues can start DMAs, and the sync/SP ring is ~10x slower than
            # the other two, so all bulk data rides scalar + gpsimd:
            #   scalar: xb0 half0, xb0 half1 (then free for the exp stream)
            #   gpsimd: twKV, twQ, [gate on h1], xb1, xb3, xb2, xb7, xb6,
            #           xb5, xb4  (+ nd outputs later)
            #   sync:   tm (small, not urgent)
            # The gate keeps xb1 from stealing HBM bandwidth from block 0.
            xb = [None] + [xpool.tile([128, BW], BF16, tag=f"xb_{cb}",
                                      name=f"xb_{cb}")
                           for cb in range(1, NQT)]
            xb0h = [xpool.tile([128, HB], BF16, tag=f"xb0h{h}", name=f"xb0h{h}")
                    for h in range(2)]
            twKV = consts.tile([128, NIC, 128], BF16, tag="twKV", name="twKV")
            twQ = consts.tile([128, NIC, 128], BF16, tag="twQ", name="twQ")
            tm = consts.tile([128, 320], BF16, tag="tm", name="tm")

            nc.gpsimd.dma_start(out=twKV, in_=wall[:, 0:NIC * 128])
            nc.scalar.dma_start(out=xb0h[0], in_=xT[:, 0:HB])
            nc.scalar.dma_start(out=xb0h[1], in_=xT[:, HB:BW])
            nc.gpsimd.dma_start(out=twQ, in_=wall[:, NIC * 128:2 * NIC * 128])
            nc.sync.dma_start(out=tm, in_=mall[:, :])
            gscr = consts.tile([128, 1], BF16, tag="gscr", name="gscr")
            nc.gpsimd.tensor_copy(gscr, xb0h[1][:, 0:1])
            for cb in (1, 3, 2, 7, 6, 5, 4):
                nc.gpsimd.dma_start(out=xb[cb],
                                    in_=xT[:, cb * BW:(cb + 1) * BW])

            tmd = tm[:, 0:128]
            tmp = tm[:, 128:256]
            tid = tm[0:DO, 256:320]

            # ---- PE p-state bridge: dummy matmuls from program start until
            # the first x data lands, so the 3us continuous-execution ramp is
            # complete when real work starts.
            dum = consts.tile([128, 128], BF16, tag="dum", name="dum")
            nc.vector.memset(dum, 0.0)
            pdum = ps_sc.tile([128, 3 * QT], F32, tag="psc", name="psc")
            for _ in range(NWARM):
                nc.tensor.matmul(pdum[:, 0:128], dum, dum, start=True, stop=True)

            zsrc = consts.tile([DO, 1], F32, tag="zsrc", name="zsrc")
            nc.vector.memset(zsrc, 0.0)
            # Dummy exp pulls the ~1.3us ACT table load off the critical path.
            zexp = consts.tile([DO, 1], F32, tag="zexp", name="zexp")
            nc.scalar.activation(zexp, zsrc,
                                 mybir.ActivationFunctionType.Exp, scale=1.0)

            def xc(ic, cb):
                return xb[cb][:, ic * QT:(ic + 1) * QT]

            def xc0(half, ic):
                return xb0h[half][:, ic * 256:(ic + 1) * 256]

            # ---- projection state ----
            # kp[i]: K^T of key tile 2i at partitions 0..63, tile 2i+1 at
            # 64..127
            kps = [qkv.tile([128, 128], BF16, tag=f"kp_{i}", name=f"kp_{i}")
                   for i in range(NOWN // 2)]
            vts = [qkv.tile([DO, QT], BF16, tag=f"vt_{st}", name=f"vt_{st}")
                   for st in range(4)]
            qts = [qkv.tile([128, QT], BF16, tag=f"qt_{st}", name=f"qt_{st}")
                   for st in range(NQT)]
            # all V1 tiles in one buffer: [128 keys, key tile, 64 V cols + 1s]
            v1big = qkv.tile([128, NOWN, DO + 1], BF16, tag="v1big",
                             name="v1big")
            nc.vector.memset(v1big[:, :, DO:DO + 1], 1.0)

            def v1(j):
                return v1big[:, j, :]

            def kv_units(st):
                """K/V projection of own key column block st, as small PE
                units; copies split across Vector and GpSimd."""
                p1 = ps_pj.tile([128, QT], F32, tag="pspj", name="pspj")
                for ic in range(0, NIC, 2):
                    def mm2(ic=ic, p1=p1):
                        nc.tensor.matmul(p1, twKV[:, ic, :], xc(ic, st),
                                         start=(ic == 0), stop=False)
                        nc.tensor.matmul(p1, twKV[:, ic + 1, :], xc(ic + 1, st),
                                         start=False, stop=(ic + 1 == NIC - 1))
                    yield mm2

                def copies(p1=p1):
                    nc.vector.tensor_copy(vts[st], p1[DO:128, :])
                    for u in range(2):
                        kp = kps[2 * st + u]
                        nc.vector.tensor_copy(kp[0:DO, :],
                                              p1[0:DO, 256 * u:256 * u + 128])
                        nc.vector.tensor_copy(kp[DO:128, :],
                                              p1[0:DO, 256 * u + 128:256 * u + 256])
                yield copies

            def tr_unit(st):
                """V transposes for block st -> v1big rows 4st..4st+3."""
                def transp():
                    pvq = ps_pj.tile([128, 4, DO], BF16, tag="pspj", name="pspj")
                    for r in range(4):
                        nc.tensor.transpose(pvq[:, r, :],
                                            vts[st][:, r * 128:r * 128 + 128],
                                            tid)
                    nc.vector.tensor_copy(v1big[:, 4 * st:4 * st + 4, 0:DO], pvq)
                yield transp

            def q_units(st):
                """Q^T (duplicated at partitions 0..63 / 64..127) for block
                st."""
                p2 = ps_pj.tile([128, QT], F32, tag="pspj", name="pspj")
                for ic in range(0, NIC, 2):
                    def mm2(ic=ic, p2=p2):
                        nc.tensor.matmul(p2, twQ[:, ic, :], xc(ic, st),
                                         start=(ic == 0), stop=False)
                        nc.tensor.matmul(p2, twQ[:, ic + 1, :], xc(ic + 1, st),
                                         start=False, stop=(ic + 1 == NIC - 1))
                    yield mm2

                def qcopy(p2=p2):
                    nc.vector.tensor_copy(qts[st], p2)
                yield qcopy

            exp_scale = float(1.0 / np.sqrt(DO))

            def mm_sc(T, j, w, sp, off):
                """One K=64 scores matmul: key tile j x last w queries of tile
                T, into sp[:, off:off+w]. Row-group from j's parity."""
                kp = kps[j // 2]
                lo = DO * (j % 2)
                nc.tensor.matmul(sp[:, off:off + w], kp[lo:lo + DO, :],
                                 qts[T][lo:lo + DO, QT - w:QT],
                                 start=True, stop=True)

            class CtxDrain:
                """Phase B for a query tile, drained a few matmuls at a time
                via the filler queue so ctx work interleaves between the next
                tile's scores groups in the in-order PE queue."""

                def __init__(self, T, ctx_args):
                    self.T = T
                    self.nk = j0_of(T) + 4
                    self.args = ctx_args
                    self.i = 0
                    self.ctxp = ps_ctx.tile([DO + 1, QT], F32, tag="ctxp",
                                            name="ctxp")

                def drain(self, n):
                    while self.i < len(self.args) and n > 0:
                        j, et_ap, qlo, w = self.args[self.i]
                        nc.tensor.matmul(self.ctxp[:, qlo:QT], v1(j),
                                         et_ap[:, 0:w],
                                         start=(j == 0), stop=(j == self.nk - 1))
                        self.i += 1
                        n -= 1

                def finish(self):
                    self.drain(len(self.args))
                    ost = ndst.tile([DO + 1, QT], F32, tag="ost", name="ost")
                    nc.vector.tensor_copy(ost, self.ctxp)
                    nc.gpsimd.dma_start(out=nd[:, self.T * QT:(self.T + 1) * QT],
                                        in_=ost)

            # ---- block-0 projections, per half: each half's matmul chain,
            # then its kp/vts/qts copies, start as soon as that half lands.
            p1 = ps_pj.tile([128, QT], F32, tag="pspj", name="pspj")
            p2 = ps_sc.tile([128, 3 * QT], F32, tag="psc", name="psc")
            for half in range(2):
                for ic in range(NIC):
                    nc.tensor.matmul(p1[:, half * 256:half * 256 + 256],
                                     twKV[:, ic, :], xc0(half, ic),
                                     start=(ic == 0), stop=(ic == NIC - 1))
                for ic in range(NIC):
                    nc.tensor.matmul(p2[:, half * 256:half * 256 + 256],
                                     twQ[:, ic, :], xc0(half, ic),
                                     start=(ic == 0), stop=(ic == NIC - 1))
                kp = kps[half]
                nc.vector.tensor_copy(kp[0:DO, :],
                                      p1[0:DO, 256 * half:256 * half + 128])
                nc.vector.tensor_copy(
                    kp[DO:128, :],
                    p1[0:DO, 256 * half + 128:256 * half + 256])
                nc.vector.tensor_copy(vts[0][:, 256 * half:256 * half + 256],
                                      p1[DO:128, 256 * half:256 * half + 256])
                nc.vector.tensor_copy(qts[0][:, 256 * half:256 * half + 256],
                                      p2[:, 256 * half:256 * half + 256])

            # ---- main loop ----
            # fillers: FIFO of (deadline, closure) proj units.  Deadlines are
            # 2*pos for units a position's full-tile scores / first ctx
            # drains depend on, 2*pos+1 for units only its band depends on.
            # The queue is pushed in nondecreasing deadline order, so a
            # front-pop flush is sufficient.
            fillers = []

            def push_units(gen, dl):
                for u in gen:
                    fillers.append((dl, u))

            def run_fillers(n):
                k = 0
                while fillers and k < n:
                    fillers.pop(0)[1]()
                    k += 1

            def flush(dl_max):
                while fillers and fillers[0][0] <= dl_max:
                    fillers.pop(0)[1]()

            # projection units per position: (st, kind, deadline).
            PROJ = {
                0: [(1, "q", 2), (0, "tr", 2), (1, "kv", 3), (1, "tr", 4)],
                1: [(3, "q", 4), (2, "kv", 4), (3, "kv", 5)],
                2: [(2, "q", 6), (3, "tr", 6), (2, "tr", 6)],
                3: [(7, "q", 8)],
                4: [(6, "q", 10)],
                5: [(5, "q", 12)],
                6: [(4, "q", 14)],
                7: [],
            }

            pending = None  # CtxDrain from the previous iteration
            for pos in range(NQT):
                T = ORD[pos]
                j0 = j0_of(T)
                mask = tmd if T < 4 else tmp
                ctx_args = []   # (j, et_ap, qlo, w) consumed via fillers

                for st, kind, dl in PROJ[pos]:
                    if kind == "q":
                        push_units(q_units(st), dl)
                    elif kind == "kv":
                        push_units(kv_units(st), dl)
                    else:
                        push_units(tr_unit(st), dl)
                # correctness: everything this position's full-tile scores or
                # first ctx drains depend on must already be emitted.
                flush(2 * pos)

                # pace leftovers + prev tile's ctx across this position's
                # full-tile slots
                nslots = -(-j0 // 3)
                per_slot = -(-len(fillers) // nslots) if nslots else 0
                dn = (-(-len(pending.args) // nslots)
                      if pending is not None and nslots else 0)

                # full key tiles 0..j0-1 in triples sharing one 3-bank PSUM
                # tile and one 1536-col exp
                j = 0
                while j < j0:
                    cnt = min(3, j0 - j)
                    sp = ps_sc.tile([128, 3 * QT], F32, tag="psc", name="psc")
                    et = expp.tile([128, 3 * QT], BF16, tag="et", name="et")
                    for u in range(cnt):
                        mm_sc(T, j + u, QT, sp, u * QT)
                    nc.scalar.activation(et[:, 0:cnt * QT], sp[:, 0:cnt * QT],
                                         mybir.ActivationFunctionType.Exp,
                                         scale=exp_scale)
                    for u in range(cnt):
                        ctx_args.append((j + u, et[:, u * QT:(u + 1) * QT],
                                         0, QT))
                    if pending is not None:
                        pending.drain(dn)
                    run_fillers(per_slot)
                    j += cnt
                # band deps (kps of the diagonal tiles)
                flush(2 * pos + 1)
                # diagonal band: all 4 tiles in ONE 3-bank tile / one exp:
                # r0 [0:512] bank1, r1 [512:896] bank2, r3 [896:1024] bank2,
                # r2 [1024:1280] bank3 (concurrent pairs hit distinct banks).
                sp = ps_sc.tile([128, 3 * QT], F32, tag="psc", name="psc")
                et = expp.tile([128, 3 * QT], BF16, tag="et", name="et")
                mm_sc(T, j0, QT, sp, 0)
                mm_sc(T, j0 + 1, 384, sp, QT)
                mm_sc(T, j0 + 2, 256, sp, 2 * QT)
                mm_sc(T, j0 + 3, 128, sp, QT + 384)
                nc.scalar.activation(et[:, 0:2 * QT + 256], sp[:, 0:2 * QT + 256],
                                     mybir.ActivationFunctionType.Exp,
                                     scale=exp_scale)
                # masks: split across Vector and GpSimd mid-kernel; all on
                # Vector at the edges (gpsimd queue busy with DMA descriptors
                # early, and the tail's ctx chains gate on them at the end).
                # Order: the tail's half-A deps (cols 0:128, 512:640) first.
                meng = nc.vector if pos in (0, NQT - 1) else nc.gpsimd
                nc.vector.tensor_mul(et[:, 0:128], et[:, 0:128], mask)
                meng.tensor_mul(et[:, QT:QT + 128], et[:, QT:QT + 128],
                                mask)
                nc.vector.tensor_mul(et[:, QT + 384:2 * QT],
                                     et[:, QT + 384:2 * QT], mask)
                meng.tensor_mul(et[:, 2 * QT:2 * QT + 128],
                                et[:, 2 * QT:2 * QT + 128], mask)
                ctx_args.append((j0, et[:, 0:QT], 0, QT))
                ctx_args.append((j0 + 1, et[:, QT:QT + 384], 128, 384))
                ctx_args.append((j0 + 2, et[:, 2 * QT:2 * QT + 256], 256, 256))
                ctx_args.append((j0 + 3, et[:, QT + 384:2 * QT], 384, 128))

                if pending is not None:
                    pending.finish()
                if j0 == 0 and pos < NQT - 1:
                    # no full-tile slots at this position: emit the next
                    # position's dependencies now, overlapping the band exp.
                    flush(2 * (pos + 1))
                if pos < NQT - 1:
                    pending = CtxDrain(T, ctx_args)
                else:
                    # ---- tail: drain the last tile's ctx in two column
                    # halves so the first half's copy+DMA overlaps the second
                    # half's matmuls.  j0 == 0 for the last tile (band only).
                    run_fillers(len(fillers))
                    ctxp = ps_ctx.tile([DO + 1, QT], F32, tag="ctxp",
                                       name="ctxp")
                    eb = et
                    H = QT // 2
                    # half A: output cols 0:256 <- tiles j0 (cols 0:256) and
                    # j0+1 (out cols 128:256 = its et cols 0:128)
                    nc.tensor.matmul(ctxp[:, 0:H], v1(0), eb[:, 0:H],
                                     start=True, stop=False)
                    nc.tensor.matmul(ctxp[:, 128:H], v1(1), eb[:, QT:QT + 128],
                                     start=False, stop=True)
                    ostA = ndst.tile([DO + 1, H], F32, tag="ost", name="ostA")
                    nc.vector.tensor_copy(ostA, ctxp[:, 0:H])
                    nc.gpsimd.dma_start(out=nd[:, T * QT:T * QT + H], in_=ostA)
                    # half B: output cols 256:512
                    ctxp2 = ps_pj.tile([DO + 1, H], F32, tag="pspj",
                                       name="ctxp2")
                    nc.tensor.matmul(ctxp2, v1(0), eb[:, H:QT],
                                     start=True, stop=False)
                    nc.tensor.matmul(ctxp2[:, 0:H], v1(1),
                                     eb[:, QT + 128:QT + 384],
                                     start=False, stop=False)
                    nc.tensor.matmul(ctxp2[:, 0:H], v1(2),
                                     eb[:, 2 * QT:2 * QT + 256],
                                     start=False, stop=False)
                    nc.tensor.matmul(ctxp2[:, 128:H], v1(3),
                                     eb[:, QT + 384:2 * QT],
                                     start=False, stop=True)
                    ostB = ndst.tile([DO + 1, H], F32, tag="ost", name="ostB")
                    nc.vector.tensor_copy(ostB, ctxp2)
                    nc.gpsimd.dma_start(out=nd[:, T * QT + H:(T + 1) * QT],
                                        in_=ostB)

    nc.compile()
    return nc


def get_program():
    if "nc" not in _prog_cache:
        _prog_cache["nc"] = build_program()
    return _prog_cache["nc"]


def core_perm(parity):
    """Permuted-to-global column index map: own key tiles first, then other."""
    own = [g for g in range(NKT) if g % 2 == parity]
    other = [g for g in range(NKT) if g % 2 != parity]
    return np.concatenate([np.arange(g * 128, (g + 1) * 128)
                           for g in own + other])


def _to_bf16(a):
    from concourse import mybir as _mybir
    return np.ascontiguousarray(a.astype(_mybir.dt.np(_mybir.dt.bfloat16)))


def make_in_maps(x, Wq, Wk, Wv):
    x = np.asarray(x, dtype=np.float32)
    Wq = np.asarray(Wq, dtype=np.float32)
    Wk = np.asarray(Wk, dtype=np.float32)
    Wv = np.asarray(Wv, dtype=np.float32)
    wkv = np.concatenate([Wk, Wv], axis=1)                     # [768, 128]
    wqq = np.concatenate([Wq, Wq], axis=1)                     # [768, 128]
    wkv_r = wkv.reshape(NIC, 128, 128).transpose(1, 0, 2).reshape(128, -1)
    wqq_r = wqq.reshape(NIC, 128, 128).transpose(1, 0, 2).reshape(128, -1)
    wall = _to_bf16(np.concatenate([wkv_r, wqq_r], axis=1))    # [128, 1536]
    mdiag = np.triu(np.ones((128, 128), dtype=np.float32))
    identp = np.concatenate([np.eye(DO, dtype=np.float32),
                             np.zeros((128 - DO, DO), np.float32)], axis=0)
    in_maps = []
    perms = []
    for c in range(NCORES):
        b, par = c // 2, c % 2
        perm = core_perm(par)
        perms.append(perm)
        xTp = x[b].T[:, perm]                                  # [768, 4096]
        # [p, block, chunk, col] layout, contiguous per partition per block;
        # block 0 is stored [p, half, chunk, 256] so its two column halves
        # are each one contiguous DMA
        blocks = xTp.reshape(NIC, 128, NQT, QT).transpose(1, 2, 0, 3)
        b0 = (blocks[:, 0].reshape(128, NIC, 2, 256).transpose(0, 2, 1, 3)
              .reshape(128, NIC * QT))
        rest = blocks[:, 1:].reshape(128, (NQT - 1) * NIC * QT)
        xr = np.concatenate([b0, rest], axis=1)
        mpcol = np.full((128, 128), 1.0 - par, dtype=np.float32)
        mall = np.concatenate([mdiag, mpcol, identp], axis=1)  # [128, 320]
        in_maps.append({
            "xT": _to_bf16(xr), "wall": wall, "mall": _to_bf16(mall),
        })
    return in_maps, perms


def combine(results, perms):
    out = np.empty((B, S, DO), dtype=np.float32)
    for b in range(B):
        num = np.zeros((DO, S), dtype=np.float64)
        den = np.zeros((S,), dtype=np.float64)
        for c in (2 * b, 2 * b + 1):
            nd_c = results[c]["nd"].astype(np.float64)
            inv = np.empty(S, dtype=np.int64)
            inv[perms[c]] = np.arange(S)
            nd_g = nd_c[:, inv]
            num += nd_g[:DO]
            den += nd_g[DO]
        out[b] = (num / den).T.astype(np.float32)
    return out


def kernel(x, Wq, Wk, Wv):
    nc = get_program()
    in_maps, perms = make_in_maps(x, Wq, Wk, Wv)
    res = run_bass_kernel_spmd(nc, in_maps, list(range(NCORES)))
    return combine(res.results, perms)
